# revision 21
# baseline (speedup 1.0000x reference)
"""Trainium2 Bass kernel for a single-head causal attention block.

Reference computation (per batch element b):
    q = X[b] @ Wq.T ; k = X[b] @ Wk.T ; v = X[b] @ Wv.T        # [S, H]
    aff = softmax(causal_mask(q @ k.T / sqrt(D)))              # [S, S]
    out[b] = aff @ v                                           # [S, H]

Sharding: data-parallel over batch — 8 batch elements, 8 NeuronCores,
one batch element per core. Weights replicated.

Per-core layout strategy ("transposed attention"):
  - Host supplies X^T [D, S] so the d-contraction sits on partitions.
  - K^T/Q^T are produced by one M=128 matmul pass (lhsT = [Wk; Wq*scale].T
    chunks), V^T by an M=64 pass; V^T is PE-transposed into [sk, 64] tiles
    augmented with a ones column (col 64).
  - Attention works on aff^T blocks [sk=128, sq=512]: exp() on ScalarE,
    causal zeroing via gpsimd affine_select, then out^T [65, 512] +=
    V_aug[k].T @ P^T accumulated over sk blocks. Row 64 of out^T is the
    softmax denominator (from the ones column).
  - QK matmuls are row-tiled: block 2p runs in PE rows 0..63 and block
    2p+1 concurrently in rows 64..127 (K^T/Q^T replicas parked at
    partitions 64..127 via DMA shifts).
  - out^T is PE-transposed back to [sq, 65]; cols 0..63 are divided by
    col 64 on the vector engine, then DMA'd out.
  - Causal structure: attention for sq-window w only needs sk blocks
    0..4w+3; fully-masked blocks are never computed.
"""

import sys

if "/opt/trn_rl_repo" not in sys.path:
    sys.path.insert(0, "/opt/trn_rl_repo")

import numpy as np

B, S, D, H = 8, 2048, 1024, 64
N_CORES = 8
W = 512           # sq window width
NW = S // W       # 4 windows
NC_ = D // 128    # 8 d-chunks
NB = S // 128     # 16 sk blocks

# Matmul dtypes: "bf16" or "f32r".
XDT_NAME = "bf16"   # X / projection weights / Q^T / K^T
PDT_NAME = "bf16"   # V tiles and exp(aff) (PV matmul operands)

N_WARMUP_MM = 7    # dummy matmuls at t=0 to lift the PE HAM throttle

_compiled = None


def _build():
    import concourse.mybir as mybir
    import concourse.tile as tile
    from concourse import bacc

    f32 = mybir.dt.float32
    xdt = getattr(mybir.dt, {"bf16": "bfloat16", "f32r": "float32r"}[XDT_NAME])
    pdt = getattr(mybir.dt, {"bf16": "bfloat16", "f32r": "float32r"}[PDT_NAME])

    nc = bacc.Bacc(None, target_bir_lowering=False)

    XT = nc.declare_dram_parameter("XT", [D, S], xdt, isOutput=False)
    WKQ = nc.declare_dram_parameter("WKQ", [D, 128], xdt, isOutput=False)
    WV = nc.declare_dram_parameter("WV", [D, H], xdt, isOutput=False)
    MSK = nc.declare_dram_parameter("MSK", [128, 896], pdt, isOutput=False)
    IDT = nc.declare_dram_parameter("IDT", [128, 128], f32, isOutput=False)
    Y = nc.declare_dram_parameter("Y", [S, H], f32, isOutput=True)

    Exp = mybir.ActivationFunctionType.Exp
    ge = mybir.AluOpType.is_ge

    with tile.TileContext(nc) as tc:
        with (
            tc.tile_pool(name="const", bufs=1) as const,
            tc.tile_pool(name="persist", bufs=1) as persist,
            tc.tile_pool(name="xt", bufs=3) as xt_pool,
            tc.tile_pool(name="evac", bufs=2) as evac_pool,
            tc.tile_pool(name="pt", bufs=4) as pt_pool,
            tc.tile_pool(name="outp", bufs=2) as out_pool,
            tc.tile_pool(name="ps_kq", bufs=1, space="PSUM") as ps_kq,
            tc.tile_pool(name="ps_vt", bufs=1, space="PSUM") as ps_vt,
            tc.tile_pool(name="ps_tr", bufs=1, space="PSUM") as ps_tr,
            tc.tile_pool(name="ps_aff", bufs=2, space="PSUM") as ps_aff,
            tc.tile_pool(name="ps_out", bufs=1, space="PSUM") as ps_out,
        ):
            # ---- PE warmup: dummy matmuls so the HAM clock-gate opens
            # while the first DMAs are in flight ----
            scratch = const.tile([128, W], xdt)
            nc.vector.memset(scratch, 0.0)
            warm = ps_aff.tile([128, 2 * W], f32, tag="aff")
            for i in range(N_WARMUP_MM):
                nc.tensor.matmul(
                    warm[:, 0:W], scratch[:, 0:128], scratch,
                    start=(i == 0), stop=(i == N_WARMUP_MM - 1),
                )
            # preload the Exp activation table while startup DMAs run
            exp_warm = const.tile([128, 2], f32)
            nc.vector.memset(exp_warm[:, 0:1], 0.0)
            nc.scalar.activation(out=exp_warm[:, 1:2], in_=exp_warm[:, 0:1], func=Exp)

            # ---- startup DMAs, critical-path first ----
            wkq_sb = const.tile([128, NC_, 128], xdt)
            nc.sync.dma_start(out=wkq_sb, in_=WKQ[:, :].rearrange("(c p) m -> p c m", p=128))

            xtr = XT[:, :].rearrange("(c p) s -> p c s", p=128)
            xt_tiles = [None] * NW
            xt_w0 = xt_pool.tile([128, NC_, W], xdt, tag="xt")
            nc.sync.dma_start(out=xt_w0[:, 0:4, :], in_=xtr[:, 0:4, 0:W])
            nc.sync.dma_start(out=xt_w0[:, 4:8, :], in_=xtr[:, 4:8, 0:W])
            xt_tiles[0] = xt_w0

            wv_sb = const.tile([128, NC_, H], xdt)
            nc.sync.dma_start(out=wv_sb, in_=WV[:, :].rearrange("(c p) m -> p c m", p=128))
            ident = const.tile([128, 128], f32)
            nc.sync.dma_start(out=ident, in_=IDT[:, :])
            mask_sb = const.tile([128, 896], pdt)
            nc.sync.dma_start(out=mask_sb, in_=MSK[:, :])

            # kq_all rows 0..63 = K^T, rows 64..127 = Q^T (as projected).
            # qT_sb = Q^T shifted down to partitions 0..63 (low-QK rhs);
            # kT_hi = K^T shifted up to partitions 64..127 (hi-QK lhsT).
            kq_all = persist.tile([128, S], xdt)
            qT_sb = persist.tile([64, S], xdt)
            kT_hi = persist.tile([128, S], xdt)
            v_aug = persist.tile([128, NB, H + 1], pdt)  # V blocks + ones col
            ones_sb = const.tile([128, 1], f32)
            nc.vector.memset(ones_sb, 1.0)
            for k in range(NB):
                nc.vector.tensor_copy(v_aug[:, k, H : H + 1], ones_sb)
            idp = const.tile([128, 128], pdt)  # identity in PV dtype
            nc.vector.tensor_copy(idp, ident)

            def proj(w):
                win = slice(w * W, (w + 1) * W)
                if xt_tiles[w] is None:
                    xt_w = xt_pool.tile([128, NC_, W], xdt, tag="xt")
                    nc.sync.dma_start(out=xt_w[:, 0:4, :], in_=xtr[:, 0:4, win])
                    nc.sync.dma_start(out=xt_w[:, 4:8, :], in_=xtr[:, 4:8, win])
                    xt_tiles[w] = xt_w
                xt_w = xt_tiles[w]

                pkq = ps_kq.tile([128, W], f32, tag="kq")
                for c in range(NC_):
                    nc.tensor.matmul(
                        pkq, wkq_sb[:, c, :], xt_w[:, c, :],
                        start=(c == 0), stop=(c == NC_ - 1),
                    )
                # K^T rows 0..63 stay on their partitions; Q^T (rows 64..127)
                # lands at partitions 64..127 (qT_hi) and is DMA-shifted down
                # to qT_sb. K^T is DMA-shifted up into kT_hi for row-tiled QK.
                nc.vector.tensor_copy(kq_all[:, win], pkq)
                nc.scalar.dma_start(out=qT_sb[:, win], in_=kq_all[64:128, win])
                nc.scalar.dma_start(out=kT_hi[64:128, win], in_=kq_all[0:64, win])

                pvt = ps_vt.tile([64, W], f32, tag="vt")
                for c in range(NC_):
                    nc.tensor.matmul(
                        pvt, wv_sb[:, c, :], xt_w[:, c, :],
                        start=(c == 0), stop=(c == NC_ - 1),
                    )
                vt_tmp = evac_pool.tile([64, W], pdt, tag="vtmp")
                nc.vector.tensor_copy(vt_tmp, pvt)
                ptr = ps_tr.tile([128, 4, H + 2], pdt, tag="tr")  # +2: keep 4B PSUM alignment
                for t in range(4):
                    nc.tensor.transpose(
                        ptr[:, t, 0:H], vt_tmp[:, t * 128 : (t + 1) * 128],
                        idp[0:64, 0:64],
                    )
                nc.vector.tensor_copy(
                    v_aug[:, 4 * w : 4 * w + 4, 0:H], ptr[:, :, 0:H]
                )

            def attn(w, pouts):
                win = slice(w * W, (w + 1) * W)
                pout = ps_out.tile([H + 1, W], f32, tag="out")
                pouts[w] = pout
                nblk = 4 * w + 4
                npairs = nblk // 2
                pts = {}

                def emit_qk_exp(p):
                    k0, k1 = 2 * p, 2 * p + 1
                    paff = ps_aff.tile([128, 2 * W], f32, tag="aff")
                    pt = pt_pool.tile([128, 2 * W], pdt, tag="pt")
                    pts[p] = pt
                    # row-tiled pair: block k0 in PE rows 0..63,
                    # block k1 concurrently in rows 64..127
                    nc.tensor.matmul(
                        paff[:, 0:W],
                        kq_all[0:64, k0 * 128 : (k0 + 1) * 128],
                        qT_sb[:, win],
                        start=True, stop=True,
                    )
                    nc.tensor.matmul(
                        paff[:, W : 2 * W],
                        kT_hi[64:128, k1 * 128 : (k1 + 1) * 128],
                        kq_all[64:128, win],
                        start=True, stop=True,
                    )
                    nc.scalar.activation(out=pt, in_=paff, func=Exp)
                    if k1 >= 4 * w:  # pair contains (partially) masked blocks
                        for h, k in ((0, k0), (1, k1)):
                            delta = 128 * k - W * w
                            if delta > -128:
                                half = pt[:, h * W : (h + 1) * W]
                                if h == 0:
                                    # zero where sk > sq via 0/1 mask multiply
                                    nc.vector.tensor_mul(
                                        half, half, mask_sb[:, 384 - delta : 896 - delta]
                                    )
                                else:
                                    # same predicate on the gpsimd engine so
                                    # the two halves mask in parallel
                                    nc.gpsimd.affine_select(
                                        out=half, in_=half,
                                        compare_op=ge, fill=0.0, base=-delta,
                                        pattern=[[1, W]], channel_multiplier=-1,
                                    )

                def emit_pv(p):
                    pt = pts.pop(p)
                    for h, k in ((0, 2 * p), (1, 2 * p + 1)):
                        nc.tensor.matmul(
                            pout,
                            v_aug[:, k, :],
                            pt[:, h * W : (h + 1) * W],
                            start=(k == 0), stop=(k == nblk - 1),
                        )

                # software pipeline: keep one QK pair in flight ahead of PV
                # so the PE never waits through the exp/mask latency
                emit_qk_exp(0)
                for p in range(1, npairs):
                    emit_qk_exp(p)
                    emit_pv(p - 1)
                emit_pv(npairs - 1)

            def attn_out(w, pouts):
                # ---------- normalize + output for window w ----------
                pout = pouts.pop(w)
                oT = out_pool.tile([H + 1, W], f32, tag="oT")
                nc.vector.tensor_copy(oT, pout)
                o_win = out_pool.tile([128, 4, H], f32, tag="osb")
                po = ps_tr.tile([128, 4, H + 1], f32, tag="tr")
                for t in range(4):
                    nc.tensor.transpose(
                        po[:, t, :], oT[:, t * 128 : (t + 1) * 128],
                        ident[0 : H + 1, 0 : H + 1],
                    )
                rec = out_pool.tile([128, 4], f32, tag="rec")
                nc.vector.reciprocal(rec, po[:, :, H])
                for t in range(4):
                    nc.vector.tensor_scalar_mul(
                        o_win[:, t, :], po[:, t, 0:H], rec[:, t : t + 1]
                    )
                yv = Y[:, :].rearrange("(w t p) h -> p (w t) h", p=128, t=4)
                if w == 0:  # runs last — split so the tail DMA is small
                    for t in range(4):
                        nc.gpsimd.dma_start(
                            out=yv[:, 4 * w + t, :], in_=o_win[:, t, :]
                        )
                else:
                    nc.gpsimd.dma_start(
                        out=yv[:, 4 * w : 4 * w + 4, :], in_=o_win
                    )

            # window 0's attention only depends on proj(0) — run it LAST so
            # the kernel tail is the shortest attention window. out(w) is
            # deferred past the next window's projections so its PE
            # transposes don't block them in the PE FIFO.
            pouts = {}
            proj(0)
            proj(1)
            attn(1, pouts)
            proj(2)
            attn_out(1, pouts)
            attn(2, pouts)
            proj(3)
            attn_out(2, pouts)
            attn(3, pouts)
            attn_out(3, pouts)
            attn(0, pouts)
            attn_out(0, pouts)

    nc.finalize()
    return nc


def _np_dt(name):
    if name == "bf16":
        import ml_dtypes

        return ml_dtypes.bfloat16
    return np.float32


def _host_inputs(X, Wk, Wq, Wv):
    """Per-core input dicts (host-side sharding + layout prep)."""
    xnp = _np_dt(XDT_NAME)
    scale = 1.0 / np.sqrt(np.float32(D))
    wkq = np.concatenate([Wk, Wq * scale], axis=0).T  # [D, 128]
    wkq = np.ascontiguousarray(wkq).astype(xnp)
    wv = np.ascontiguousarray(Wv.T).astype(xnp)  # [D, H]
    idt = np.eye(128, dtype=np.float32)

    # multiplicative causal mask master strip: M[p, c] = 1 iff (c - p) >= 384
    pnp = _np_dt(PDT_NAME)
    pp = np.arange(128)[:, None]
    cc = np.arange(896)[None, :]
    msk = ((cc - pp) >= 384).astype(pnp)

    in_maps = []
    for b in range(N_CORES):
        xt = np.ascontiguousarray(X[b].T).astype(xnp)  # [D, S]
        in_maps.append({"XT": xt, "WKQ": wkq, "WV": wv, "MSK": msk, "IDT": idt})
    return in_maps


def kernel(X, Wk, Wq, Wv):
    global _compiled
    from concourse.bass_utils import run_bass_kernel_spmd

    if _compiled is None:
        _compiled = _build()
    in_maps = _host_inputs(
        np.asarray(X, dtype=np.float32),
        np.asarray(Wk, dtype=np.float32),
        np.asarray(Wq, dtype=np.float32),
        np.asarray(Wv, dtype=np.float32),
    )
    res = run_bass_kernel_spmd(_compiled, in_maps, list(range(N_CORES)))
    out = np.stack([res.results[i]["Y"] for i in range(N_CORES)], axis=0)
    return out.astype(np.float32)
